# revision 1
# baseline (speedup 1.0000x reference)
import sys, os
sys.path.insert(0, "/opt/trn_rl_repo")
import numpy as np
import ml_dtypes

DIM = 256; DIM_HEAD = 32; HEADS = 8; WSZ = 8; D4 = 64
EPS = 1e-5
SCALE = DIM_HEAD ** -0.5
NCORES = 8
HSH = 32  # H rows per core (256/8)
BF16 = ml_dtypes.bfloat16


def _ln_np(x, g, b):
    m = x.mean(-1, keepdims=True)
    v = x.var(-1, keepdims=True)
    return (x - m) / np.sqrt(v + EPS) * g + b


def _dpb_bias64(dpb_w1, dpb_b1, dpb_g1, dpb_beta1,
                dpb_w2, dpb_b2, dpb_g2, dpb_beta2,
                dpb_w3, dpb_b3, dpb_g3, dpb_beta3,
                dpb_w4, dpb_b4):
    pos = np.arange(-WSZ, WSZ + 1, dtype=np.float32)
    rel = np.stack(np.meshgrid(pos, pos, indexing='ij')).reshape(2, -1).T
    h = np.maximum(_ln_np(rel @ dpb_w1.T + dpb_b1, dpb_g1, dpb_beta1), 0)
    h = np.maximum(_ln_np(h @ dpb_w2.T + dpb_b2, dpb_g2, dpb_beta2), 0)
    h = np.maximum(_ln_np(h @ dpb_w3.T + dpb_b3, dpb_g3, dpb_beta3), 0)
    biases = (h @ dpb_w4.T + dpb_b4)[:, 0]
    p = np.arange(WSZ)
    grid = np.stack(np.meshgrid(p, p, indexing='ij')).reshape(2, -1).T
    r = grid[:, None] - grid[None, :] + WSZ - 1
    idx = r[..., 0] * (2 * WSZ - 1) + r[..., 1]
    return biases[idx].astype(np.float32)  # (64, 64)


def _build_nc():
    from contextlib import ExitStack
    import concourse.bass as bass
    import concourse.tile as tile
    from concourse import mybir
    from concourse.tile import TileContext

    f32 = mybir.dt.float32
    bf16 = mybir.dt.bfloat16
    AX = mybir.AxisListType.X
    AF = mybir.ActivationFunctionType

    nc = bass.Bass()
    x_e = nc.declare_dram_parameter("x", [2, DIM, HSH, 256], f32, isOutput=False)
    wqkvT_e = nc.declare_dram_parameter("wqkvT", [2, 128, 768], bf16, isOutput=False)
    bq_e = nc.declare_dram_parameter("bq", [6, 128, 1], f32, isOutput=False)
    woutT_e = nc.declare_dram_parameter("woutT", [2, 128, 256], bf16, isOutput=False)
    bo_e = nc.declare_dram_parameter("bo", [2, 128, 1], f32, isOutput=False)
    bias_e = nc.declare_dram_parameter("biasmat", [64, 512], f32, isOutput=False)
    idf_e = nc.declare_dram_parameter("idf", [128, 128], f32, isOutput=False)
    idb_e = nc.declare_dram_parameter("idb", [128, 128], bf16, isOutput=False)
    out_e = nc.declare_dram_parameter("out", [2, DIM, HSH, 256], f32, isOutput=True)

    with TileContext(nc) as tc, ExitStack() as ctx:
        cpool = ctx.enter_context(tc.tile_pool(name="consts", bufs=1))
        # persistent weights in SBUF
        wqkvT = [cpool.tile([128, 768], bf16, tag=f"wq{i}", name=f"wq{i}") for i in range(2)]
        woutT = [cpool.tile([128, 256], bf16, tag=f"wo{i}", name=f"wo{i}") for i in range(2)]
        bq = [cpool.tile([128, 1], f32, tag=f"bq{i}", name=f"bq{i}") for i in range(6)]
        bo = [cpool.tile([128, 1], f32, tag=f"bo{i}", name=f"bo{i}") for i in range(2)]
        biasm = cpool.tile([64, 512], f32, tag="biasm", name="biasm")
        idf = cpool.tile([128, 128], f32, tag="idf", name="idf")
        idb = cpool.tile([128, 128], bf16, tag="idb", name="idb")
        for i in range(2):
            nc.sync.dma_start(out=wqkvT[i][:], in_=wqkvT_e[i])
            nc.sync.dma_start(out=woutT[i][:], in_=woutT_e[i])
            nc.sync.dma_start(out=bo[i][:], in_=bo_e[i])
        for i in range(6):
            nc.sync.dma_start(out=bq[i][:], in_=bq_e[i])
        nc.sync.dma_start(out=biasm[:], in_=bias_e[:])
        nc.sync.dma_start(out=idf[:], in_=idf_e[:])
        nc.sync.dma_start(out=idb[:], in_=idb_e[:])

        xpool = ctx.enter_context(tc.tile_pool(name="xp", bufs=2))
        tpool = ctx.enter_context(tc.tile_pool(name="tp", bufs=2))
        qpool = ctx.enter_context(tc.tile_pool(name="qp", bufs=2))
        apool = ctx.enter_context(tc.tile_pool(name="ap", bufs=2))
        opool = ctx.enter_context(tc.tile_pool(name="op", bufs=2))
        p_tr = ctx.enter_context(tc.tile_pool(name="ptr", bufs=1, space="PSUM"))
        p_mm = ctx.enter_context(tc.tile_pool(name="pmm", bufs=2, space="PSUM"))
        p_sim = ctx.enter_context(tc.tile_pool(name="psim", bufs=2, space="PSUM"))
        p_ao = ctx.enter_context(tc.tile_pool(name="pao", bufs=2, space="PSUM"))

        for b in range(2):
            for hb in range(4):          # window-row: rows hb*8 .. hb*8+8
                for ws in range(4):      # sub-strip: 8 windows, cols ws*64..+64
                    h0 = hb * 8
                    w0 = ws * 64
                    # ---- load x rows contiguously, then reorder to window-major on-chip
                    xr = [xpool.tile([128, 512], f32, tag=f"xr{c}", name=f"xr{c}") for c in range(2)]
                    xv = [xpool.tile([128, 512], f32, tag=f"xv{c}", name=f"xv{c}") for c in range(2)]
                    xt = [xpool.tile([128, 512], f32, tag=f"x{c}", name=f"x{c}") for c in range(2)]
                    for c in range(2):
                        src = x_e[b, c * 128:(c + 1) * 128, h0:h0 + 8, w0:w0 + 64]
                        nc.sync.dma_start(out=xr[c][:].rearrange("c (s1 w) -> c s1 w", s1=8), in_=src)
                        # row-major (s1, ww, s2) -> window-major (ww, s1, s2) on DVE
                        sv = xr[c][:].rearrange("c (s1 ww s2) -> c s1 ww s2", s1=8, ww=8)
                        dv = xv[c][:].rearrange("c (ww s1 s2) -> c s1 ww s2", ww=8, s1=8)
                        nc.vector.tensor_copy(dv, sv)
                        # wash through DMA so the transpose (LDW) depends on a DMA, not DVE
                        nc.sync.dma_start(out=xt[c][:], in_=xv[c][:])
                    # ---- transpose to [tok, c] and LayerNorm
                    xh = [xpool.tile([128, 256], f32, tag=f"xh{t}", name=f"xh{t}") for t in range(4)]
                    for t in range(4):
                        for c in range(2):
                            ps = p_tr.tile([128, 128], f32, tag="tr", name="tr")
                            nc.tensor.transpose(ps[:], xt[c][:, t * 128:(t + 1) * 128], idf[:])
                            nc.scalar.copy(xh[t][:, c * 128:(c + 1) * 128], ps[:])
                    xhc = [xpool.tile([128, 512], bf16, tag=f"xhc{c}", name=f"xhc{c}") for c in range(2)]
                    for t in range(4):
                        mn = tpool.tile([128, 1], f32, tag="mn", name="mn")
                        nc.vector.reduce_sum(mn[:], xh[t][:], axis=AX)
                        nc.scalar.activation(mn[:], mn[:], AF.Copy, scale=-1.0 / 256)
                        xc = tpool.tile([128, 256], f32, tag="xc", name="xc")
                        nc.vector.tensor_scalar_add(xc[:], xh[t][:], mn[:])
                        sq = tpool.tile([128, 256], f32, tag="sq", name="sq")
                        nc.vector.tensor_mul(sq[:], xc[:], xc[:])
                        vr = tpool.tile([128, 1], f32, tag="vr", name="vr")
                        nc.vector.reduce_sum(vr[:], sq[:], axis=AX)
                        nc.scalar.activation(vr[:], vr[:], AF.Sqrt, scale=1.0 / 256)
                        rs = tpool.tile([128, 1], f32, tag="rs", name="rs")
                        nc.vector.reciprocal(rs[:], vr[:])
                        nc.vector.tensor_scalar_mul(xc[:], xc[:], rs[:])
                        xcd = tpool.tile([128, 256], f32, tag="xcd", name="xcd")
                        nc.sync.dma_start(out=xcd[:], in_=xc[:])
                        # transpose back to [c, tok], cast bf16
                        for c in range(2):
                            ps = p_tr.tile([128, 128], f32, tag="tr", name="tr")
                            nc.tensor.transpose(ps[:], xcd[:, c * 128:(c + 1) * 128], idf[:])
                            nc.scalar.copy(xhc[c][:, t * 128:(t + 1) * 128], ps[:])
                    # ---- QKV projection (g, scale, b folded on host)
                    qkv = [qpool.tile([64, 512], bf16, tag=f"qkv{e}", name=f"qkv{e}") for e in range(12)]
                    for e in range(6):
                        ps = p_mm.tile([128, 512], f32, tag="mm", name="mm")
                        nc.tensor.matmul(ps[:], wqkvT[0][:, e * 128:(e + 1) * 128], xhc[0][:], start=True, stop=False)
                        nc.tensor.matmul(ps[:], wqkvT[1][:, e * 128:(e + 1) * 128], xhc[1][:], start=False, stop=True)
                        if e < 2 or e >= 4:
                            qtmp = qpool.tile([128, 512], bf16, tag=f"qt{e}", name=f"qt{e}")
                            nc.vector.tensor_scalar_add(qtmp[0:64, :], ps[0:64, :], bq[e][0:64, :])
                            nc.vector.tensor_scalar_add(qtmp[64:128, :], ps[64:128, :], bq[e][64:128, :])
                            nc.sync.dma_start(out=qkv[2 * e][:], in_=qtmp[0:64, :])
                            nc.sync.dma_start(out=qkv[2 * e + 1][:], in_=qtmp[64:128, :])
                        else:
                            nc.vector.tensor_scalar_add(qkv[2 * e][:], ps[0:64, :], bq[e][0:64, :])
                            nc.vector.tensor_scalar_add(qkv[2 * e + 1][:], ps[64:128, :], bq[e][64:128, :])
                    # ---- attention per window
                    ao = [apool.tile([128, 512], bf16, tag=f"ao{c}", name=f"ao{c}") for c in range(2)]
                    for w in range(8):
                        sl = slice(w * 64, w * 64 + 64)
                        simp = p_sim.tile([64, 512], f32, tag="sim", name="sim")
                        for h in range(HEADS):
                            p, l = h // 2, (h % 2) * 32
                            nc.tensor.matmul(simp[:, h * 64:(h + 1) * 64],
                                             qkv[p][l:l + 32, sl],
                                             qkv[4 + p][l:l + 32, sl],
                                             start=True, stop=True)
                        ee = apool.tile([64, 512], f32, tag="ee", name="ee")
                        nc.vector.tensor_add(ee[:], simp[:], biasm[:])
                        nc.scalar.activation(ee[:], ee[:], AF.Exp)
                        aa = apool.tile([64, 512], bf16, tag="aa", name="aa")
                        for h in range(HEADS):
                            dn = tpool.tile([64, 1], f32, tag="dn", name="dn")
                            nc.vector.reduce_sum(dn[:], ee[:, h * 64:(h + 1) * 64], axis=AX)
                            rc = tpool.tile([64, 1], f32, tag="rc", name="rc")
                            nc.vector.reciprocal(rc[:], dn[:])
                            nc.vector.tensor_scalar_mul(aa[:, h * 64:(h + 1) * 64],
                                                        ee[:, h * 64:(h + 1) * 64], rc[:])
                        aaw = apool.tile([64, 512], bf16, tag="aaw", name="aaw")
                        nc.sync.dma_start(out=aaw[:], in_=aa[:])
                        # vT: [64 j, 256] pair-major (4 pairs x 64)
                        vT = apool.tile([64, 256], bf16, tag="vT", name="vT")
                        for p in range(4):
                            ps = p_tr.tile([128, 128], bf16, tag="trb", name="trb")
                            nc.tensor.transpose(ps[:64, :64], qkv[8 + p][:, sl], idb[0:64, 0:64])
                            nc.scalar.copy(vT[:, p * 64:(p + 1) * 64], ps[:64, :64])
                        # AT per head: [64 j, 64 i]
                        aT = apool.tile([64, 512], bf16, tag="aT", name="aT")
                        for h in range(HEADS):
                            ps = p_tr.tile([128, 128], bf16, tag="trb", name="trb")
                            nc.tensor.transpose(ps[:64, :64], aaw[:, h * 64:(h + 1) * 64], idb[0:64, 0:64])
                            nc.scalar.copy(aT[:, h * 64:(h + 1) * 64], ps[:64, :64])
                        vTw = apool.tile([64, 256], bf16, tag="vTw", name="vTw")
                        nc.sync.dma_start(out=vTw[:], in_=vT[:])
                        # out2 = vT_h^T @ aT_h -> [32 d, 64 i], pairs of heads
                        for p in range(4):
                            pa = p_ao.tile([64, 64], f32, tag="pao", name="pao")
                            for l in range(2):
                                h = p * 2 + l
                                nc.tensor.matmul(pa[l * 32:(l + 1) * 32, :],
                                                 vTw[:, p * 64 + l * 32:p * 64 + l * 32 + 32],
                                                 aT[:, h * 64:(h + 1) * 64],
                                                 start=True, stop=True)
                            nc.vector.tensor_copy(ao[p // 2][(p % 2) * 64:(p % 2) * 64 + 64, sl], pa[:])
                    # ---- output projection
                    ot = [opool.tile([128, 512], f32, tag=f"ot{c}", name=f"ot{c}") for c in range(2)]
                    for c in range(2):
                        ps = p_mm.tile([128, 512], f32, tag="mm", name="mm")
                        nc.tensor.matmul(ps[:], woutT[0][:, c * 128:(c + 1) * 128], ao[0][:], start=True, stop=False)
                        nc.tensor.matmul(ps[:], woutT[1][:, c * 128:(c + 1) * 128], ao[1][:], start=False, stop=True)
                        nc.vector.tensor_scalar_add(ot[c][:], ps[:], bo[c][:])
                        # window-major -> row-major, then contiguous store
                        orm = opool.tile([128, 512], f32, tag=f"orm{c}", name=f"orm{c}")
                        sv = ot[c][:].rearrange("c (ww s1 s2) -> c s1 ww s2", ww=8, s1=8)
                        dv = orm[:].rearrange("c (s1 ww s2) -> c s1 ww s2", s1=8, ww=8)
                        nc.vector.tensor_copy(dv, sv)
                        dst = out_e[b, c * 128:(c + 1) * 128, h0:h0 + 8, w0:w0 + 64]
                        nc.sync.dma_start(out=dst, in_=orm[:].rearrange("c (s1 w) -> c s1 w", s1=8))
    return nc


LAST = None


def _patch_ldw_opt():
    # work around a walrus codegen crash (setupSyncWait on S3_LW) seen with
    # --enable-ldw-opt=false: flip the flag on the compile command line.
    import concourse.bass_utils as _bu
    if getattr(_bu, "_ldw_patched", False):
        return
    _orig_rc = _bu.run_command

    def _patched_rc(cmd, *a, **k):
        if isinstance(cmd, list):
            cmd = ["--enable-ldw-opt=true" if c == "--enable-ldw-opt=false" else c
                   for c in cmd]
        return _orig_rc(cmd, *a, **k)

    _bu.run_command = _patched_rc
    _bu._ldw_patched = True


def _kernel_bass(x, wqkvT, bq, woutT, bo, biasmat, idf, idb):
    global LAST
    from concourse.bass_utils import run_bass_kernel_spmd
    _patch_ldw_opt()
    nc = _build_nc()
    in_maps = []
    for i in range(NCORES):
        xs = np.ascontiguousarray(x[:, :, i * HSH:(i + 1) * HSH, :])
        in_maps.append(dict(x=xs, wqkvT=wqkvT, bq=bq, woutT=woutT, bo=bo,
                            biasmat=biasmat, idf=idf, idb=idb))
    res = run_bass_kernel_spmd(nc, in_maps, core_ids=list(range(NCORES)))
    LAST = res
    out = np.empty((2, DIM, 256, 256), dtype=np.float32)
    for i in range(NCORES):
        out[:, :, i * HSH:(i + 1) * HSH, :] = res.results[i]["out"]
    return out


def _prep(x, norm_g, norm_b, w_qkv, w_out, b_out, **dpb):
    g = np.asarray(norm_g, np.float32).reshape(DIM)
    bvec = np.asarray(norm_b, np.float32).reshape(DIM)
    wq = np.asarray(w_qkv, np.float32) * g[None, :]
    bqv = np.asarray(w_qkv, np.float32) @ bvec
    wq[:256] *= SCALE
    bqv = bqv.copy(); bqv[:256] *= SCALE
    wqkvT = np.ascontiguousarray(wq.T.reshape(2, 128, 768)).astype(BF16)
    bq = np.ascontiguousarray(bqv.reshape(6, 128, 1)).astype(np.float32)
    woutT = np.ascontiguousarray(np.asarray(w_out, np.float32).T.reshape(2, 128, 256)).astype(BF16)
    bo = np.ascontiguousarray(np.asarray(b_out, np.float32).reshape(2, 128, 1))
    bias64 = _dpb_bias64(**{k: np.asarray(v, np.float32) for k, v in dpb.items()})
    biasmat = np.ascontiguousarray(np.tile(bias64, (1, 8)))
    idf = np.eye(128, dtype=np.float32)
    idb = np.eye(128).astype(BF16)
    return wqkvT, bq, woutT, bo, biasmat, idf, idb


def _kernel_numpy(x, norm_g, norm_b, w_qkv, w_out, b_out, **dpb):
    # fallback: straight port of the reference in numpy (f32)
    B, D, H, W = x.shape
    nh, nw = H // WSZ, W // WSZ
    mean = x.mean(axis=1, keepdims=True)
    var = x.var(axis=1, keepdims=True)
    xn = (x - mean) / np.sqrt(var + EPS) * norm_g + norm_b
    xw = xn.reshape(B, D, nh, WSZ, nw, WSZ).transpose(0, 2, 4, 1, 3, 5)
    xw = xw.reshape(B * nh * nw, D, WSZ * WSZ)
    qkv = np.einsum('ed,bdn->ben', w_qkv, xw)
    q, k, v = np.split(qkv, 3, axis=1)
    th = lambda t: t.reshape(-1, HEADS, DIM_HEAD, WSZ * WSZ).transpose(0, 1, 3, 2)
    q, k, v = th(q) * SCALE, th(k), th(v)
    sim = np.einsum('bhid,bhjd->bhij', q, k)
    sim = sim + _dpb_bias64(**dpb)[None, None]
    sim = sim - sim.max(-1, keepdims=True)
    e = np.exp(sim)
    attn = e / e.sum(-1, keepdims=True)
    o = np.einsum('bhij,bhjd->bhid', attn, v)
    o = o.transpose(0, 1, 3, 2).reshape(-1, HEADS * DIM_HEAD, WSZ * WSZ)
    o = np.einsum('de,ben->bdn', w_out, o) + b_out[None, :, None]
    o = o.reshape(B, nh, nw, D, WSZ, WSZ).transpose(0, 3, 1, 4, 2, 5).reshape(B, D, H, W)
    return o.astype(np.float32)


def kernel(**inputs):
    inputs = {k: np.asarray(v) for k, v in inputs.items()}
    try:
        pre = _prep(**inputs)
        return _kernel_bass(np.asarray(inputs["x"], np.float32), *pre)
    except Exception as ex:
        sys.stderr.write(f"[kernel] bass path failed ({ex!r}); numpy fallback\n")
        return _kernel_numpy(**inputs)

